# revision 2
# baseline (speedup 1.0000x reference)
"""CMFM loss kernel for Trainium2 (8 NeuronCores, Bass/Tile) — v2.

Math: for inputs f_v, f_a [B,T,D] with vn/an the D-normalized tensors,
  cos[b,t]    = s_va * inv_v * inv_a       (per-timestep term)
  sum_{i!=j} cross = (1/T)*(sum_t V_t.A_t - sum_{b,t} cos[b,t])
where V_t = sum_b vn[b,t,:].  Data-parallel over B (8 rows/core); host
sums the 8 per-core V/A partials and dots them.

v2 design (all costs HW-measured on this stack):
  * bf16 everywhere on-chip: SWDGE casting DMAs (fp32 HBM -> bf16 SBUF)
    at 2MB granularity run at ~440GB/s, vs ~350GB/s for plain HWDGE.
  * TensorE does the normalize-and-accumulate: per (b,tc) a diagonal
    stationary diag(inv) [128x128] bf16 times the moving data tile
    [128,256] accumulates V_t += inv_v*v directly in PSUM (124ns/chunk
    incl. weight load).  V uses 4 PSUM banks, A the other 4; each bank
    is has_written-cleared once by a zero-stationary dummy matmul (the
    start=True clear is bank-wide, so half-bank groups can't use it).
  * Diagonals are built 8-at-a-time with a double-broadcast tensor op
    (ident[p,f] * inv[p,c]): DVE for v (1.2us/row), GpSimd for a
    (the Q7s otherwise only do SWDGE emission).
  * s_vv/s_aa: ACT batched Square (257ns/tile-eq) + DVE halve-add (2x
    mode) + tensor_reduce; a knob moves trailing chunks to fused
    ACT Square+accum (750ns/chunk) to balance ACT vs DVE.
  * s_va: DVE TT mult (2x, 161ns/tile-eq) + halve + reduce.
Measured per-op: DVE TT bf16 161/t-e, bcast-TT 279/t-e (1x), reduce
276/t-e, ACT batched sq 257/t-e, fused sq+accum 750/op, PE diag-MM
124/op, gps TT-bcast 414/t-e.
"""

import os as _os

import numpy as np

import concourse.bacc as bacc
import concourse.tile as tile
from concourse import mybir
from concourse.bass_utils import run_bass_kernel_spmd

ALPHA, BETA, GAMMA = 2.0, 2.0, 1.0
B, T, D = 64, 1024, 256
N_CORES = 8
B_LOC = B // N_CORES          # 8 batch rows per core
P = 128                       # SBUF partitions
TCH = T // P                  # 8 t-chunks per batch row
NTILES = B_LOC * TCH          # 64 (b,tc) tiles per core
HD = D // 2                   # 128, halve-add width

F32 = mybir.dt.float32
BF16 = mybir.dt.bfloat16
MULT = mybir.AluOpType.mult
ADD = mybir.AluOpType.add

# knobs
K_FV = int(_os.environ.get("K_FV", "2"))   # fused ACT sq+accum chunks/row (v)
K_FA = int(_os.environ.get("K_FA", "2"))   # fused chunks/row (a)
K_DIAG = _os.environ.get("K_DIAG", "split")  # 'dve' | 'gps' | 'split'
K_IO_BUFS = int(_os.environ.get("K_IO_BUFS", "3"))
K_SCR_BUFS = int(_os.environ.get("K_SCR_BUFS", "2"))

_CACHE = {}
LAST_RESULTS = None


def _build_nc(repeat=1, loop_n=1):
    nc = bacc.Bacc("TRN2", debug=False)

    v = nc.dram_tensor("v", [B_LOC, T, D], F32, kind="ExternalInput").ap()
    a = nc.dram_tensor("a", [B_LOC, T, D], F32, kind="ExternalInput").ap()
    cos_out = nc.dram_tensor("cos_stat", [P, NTILES], F32, kind="ExternalOutput").ap()
    vacc_out = nc.dram_tensor("v_acc", [P, TCH * D], F32, kind="ExternalOutput").ap()
    aacc_out = nc.dram_tensor("a_acc", [P, TCH * D], F32, kind="ExternalOutput").ap()

    with tile.TileContext(nc) as tc:
        with (
            tc.tile_pool(name="io", bufs=K_IO_BUFS) as iop,
            tc.tile_pool(name="scr", bufs=K_SCR_BUFS) as scr,
            tc.tile_pool(name="small", bufs=K_SCR_BUFS + 1) as small,
            tc.tile_pool(name="const", bufs=1) as cst,
            tc.tile_pool(name="ps", bufs=1, space="PSUM") as psp,
        ):
            # --- persistent/constant tiles ---
            ident = cst.tile([P, P], BF16)
            zeros512 = cst.tile([P, 4, HD], BF16)   # moving operand for dummy-clears
            sva_stat = cst.tile([P, NTILES], F32)
            ivp_stat = cst.tile([P, NTILES], F32)   # inv_v*inv_a per (b,tc)
            cos_stat = cst.tile([P, NTILES], F32)
            psV = psp.tile([P, TCH, D], F32)        # 4 banks
            psA = psp.tile([P, TCH, D], F32)        # 4 banks

            iota_p = cst.tile([P, 1], F32)
            iota_f = cst.tile([P, P], F32)
            nc.gpsimd.iota(iota_p[:], pattern=[[1, 1]], base=0,
                           channel_multiplier=1,
                           allow_small_or_imprecise_dtypes=True)
            nc.gpsimd.iota(iota_f[:], pattern=[[1, P]], base=0,
                           channel_multiplier=0,
                           allow_small_or_imprecise_dtypes=True)
            nc.vector.tensor_scalar(out=ident[:], in0=iota_f[:],
                                    scalar1=iota_p[:, 0:1], scalar2=None,
                                    op0=mybir.AluOpType.is_equal)
            nc.gpsimd.memset(zeros512[:], 0.0)

            ident_bc = ident[:].rearrange("p (o q) -> p o q", o=1) \
                               .broadcast_to([P, TCH, P])

            import contextlib
            loop_ctx = (
                tc.For_i(
                    0, loop_n, 1,
                    hint_engines=(
                        mybir.EngineType.DVE,
                        mybir.EngineType.Activation,
                        mybir.EngineType.SP,
                        mybir.EngineType.PE,
                    ),
                )
                if loop_n > 1
                else contextlib.nullcontext()
            )
            with loop_ctx:
              for _ in range(repeat):
                # dummy matmuls: zero stationary, start=True clears each
                # bank's has_written bits so the per-chunk groups accumulate
                for k in range(4):
                    nc.tensor.matmul(out=psV[:, 2 * k:2 * k + 2, :],
                                     lhsT=zeros512[:, 0, :], rhs=zeros512[:],
                                     start=True, stop=False,
                                     skip_group_check=True)
                    nc.tensor.matmul(out=psA[:, 2 * k:2 * k + 2, :],
                                     lhsT=zeros512[:, 0, :], rhs=zeros512[:],
                                     start=True, stop=False,
                                     skip_group_check=True)

                supers = None
                for b in range(B_LOC):
                    if b % 2 == 0:
                        vt2 = iop.tile([P, 2, TCH, D], BF16, tag="vt")
                        at2 = iop.tile([P, 2, TCH, D], BF16, tag="at")
                        vr = v[b:b + 2].rearrange("b (j p) d -> p b j d", p=P)
                        ar = a[b:b + 2].rearrange("b (j p) d -> p b j d", p=P)
                        nc.gpsimd.dma_start(out=vt2[:], in_=vr)
                        nc.gpsimd.dma_start(out=at2[:], in_=ar)
                        supers = (vt2, at2)
                    vt2, at2 = supers
                    vt = vt2[:, b % 2]     # [P, TCH, D] bf16
                    at = at2[:, b % 2]

                    pair = small.tile([P, 2 * TCH], F32, tag="pair")
                    dump = scr.tile([P, D], BF16, tag="dump")

                    # --- s_vv / s_aa ---
                    for (x, nf, col) in ((vt, K_FV, 0), (at, K_FA, 1)):
                        nb = TCH - nf      # batched chunks [0, nb)
                        sq = scr.tile([P, TCH, D], BF16, tag=f"sq{col}")
                        if nb > 0:
                            nc.scalar.activation(
                                out=sq[:, 0:nb, :], in_=x[:, 0:nb, :],
                                func=mybir.ActivationFunctionType.Square)
                            hv = scr.tile([P, TCH, HD], BF16, tag=f"hv{col}")
                            nc.vector.tensor_tensor(
                                out=hv[:, 0:nb, :], in0=sq[:, 0:nb, 0:HD],
                                in1=sq[:, 0:nb, HD:D], op=ADD)
                            nc.vector.tensor_reduce(
                                out=pair[:, col * TCH:col * TCH + nb],
                                in_=hv[:, 0:nb, :],
                                axis=mybir.AxisListType.X, op=ADD)
                        for c in range(nb, TCH):
                            nc.scalar.activation(
                                out=dump[:], in_=x[:, c, :],
                                func=mybir.ActivationFunctionType.Square,
                                accum_out=pair[:, col * TCH + c:col * TCH + c + 1])

                    # --- s_va ---
                    prod = scr.tile([P, TCH, D], BF16, tag="prod")
                    nc.vector.tensor_tensor(out=prod[:], in0=vt, in1=at,
                                            op=MULT)
                    hp = scr.tile([P, TCH, HD], BF16, tag="hp")
                    nc.vector.tensor_tensor(out=hp[:], in0=prod[:, :, 0:HD],
                                            in1=prod[:, :, HD:D], op=ADD)
                    nc.vector.tensor_reduce(
                        out=sva_stat[:, b * TCH:(b + 1) * TCH], in_=hp[:],
                        axis=mybir.AxisListType.X, op=ADD)

                    # --- norms -> inv (bf16 for diags, f32 product for cos)
                    norm = small.tile([P, 2 * TCH], F32, tag="norm")
                    inv_f = small.tile([P, 2 * TCH], F32, tag="invf")
                    inv_b = small.tile([P, 2 * TCH], BF16, tag="invb")
                    nc.scalar.activation(out=norm[:], in_=pair[:],
                                         func=mybir.ActivationFunctionType.Sqrt)
                    nc.vector.reciprocal(out=inv_f[:], in_=norm[:])
                    nc.vector.tensor_copy(out=inv_b[:], in_=inv_f[:])
                    nc.vector.tensor_mul(
                        out=ivp_stat[:, b * TCH:(b + 1) * TCH],
                        in0=inv_f[:, 0:TCH], in1=inv_f[:, TCH:2 * TCH])

                    # --- diagonals: dg[p, c, f] = ident[p, f] * inv[p, c]
                    dgv = scr.tile([P, TCH, P], BF16, tag="dgv")
                    dga = scr.tile([P, TCH, P], BF16, tag="dga")
                    ibv = inv_b[:].rearrange("p (k c) -> p k c", k=2)[:, 0] \
                        .rearrange("p (c o) -> p c o", o=1) \
                        .broadcast_to([P, TCH, P])
                    iba = inv_b[:].rearrange("p (k c) -> p k c", k=2)[:, 1] \
                        .rearrange("p (c o) -> p c o", o=1) \
                        .broadcast_to([P, TCH, P])
                    dg_v_eng = nc.vector if K_DIAG in ("dve", "split") else nc.gpsimd
                    dg_a_eng = nc.gpsimd if K_DIAG in ("gps", "split") else nc.vector
                    dg_v_eng.tensor_tensor(out=dgv[:], in0=ident_bc, in1=ibv,
                                           op=MULT)
                    dg_a_eng.tensor_tensor(out=dga[:], in0=ident_bc, in1=iba,
                                           op=MULT)

                    # --- PE: V/A accumulate, per chunk ---
                    last = (b == B_LOC - 1)
                    for c in range(TCH):
                        nc.tensor.matmul(out=psV[:, c, :], lhsT=dgv[:, c, :],
                                         rhs=vt[:, c, :], start=False,
                                         stop=last, skip_group_check=True)
                        nc.tensor.matmul(out=psA[:, c, :], lhsT=dga[:, c, :],
                                         rhs=at[:, c, :], start=False,
                                         stop=last, skip_group_check=True)

                # --- epilogue ---
                nc.vector.tensor_mul(out=cos_stat[:], in0=sva_stat[:],
                                     in1=ivp_stat[:])
                nc.sync.dma_start(out=cos_out, in_=cos_stat[:])
                vacc_sb = cst.tile([P, TCH, D], F32)
                aacc_sb = cst.tile([P, TCH, D], F32)
                nc.vector.tensor_copy(out=vacc_sb[:], in_=psV[:])
                nc.vector.tensor_copy(out=aacc_sb[:], in_=psA[:])
                nc.sync.dma_start(out=vacc_out, in_=vacc_sb[:])
                nc.sync.dma_start(out=aacc_out, in_=aacc_sb[:])

    nc.compile()
    return nc


def _get_nc(repeat=1, loop_n=1):
    key = ("nc", repeat, loop_n, K_FV, K_FA, K_DIAG, K_IO_BUFS, K_SCR_BUFS)
    if key not in _CACHE:
        _CACHE[key] = _build_nc(repeat, loop_n)
    return _CACHE[key]


def _run(nc, f_v, f_a):
    in_maps = [
        {
            "v": np.ascontiguousarray(f_v[c * B_LOC:(c + 1) * B_LOC]),
            "a": np.ascontiguousarray(f_a[c * B_LOC:(c + 1) * B_LOC]),
        }
        for c in range(N_CORES)
    ]
    return run_bass_kernel_spmd(nc, in_maps, core_ids=list(range(N_CORES)))


def kernel(f_v, f_a, labels):
    global LAST_RESULTS
    f_v = np.asarray(f_v, dtype=np.float32)
    f_a = np.asarray(f_a, dtype=np.float32)
    labels = np.asarray(labels)

    res = _run(_get_nc(), f_v, f_a)
    LAST_RESULTS = res
    out = res.results

    # cos_stat[c][p, b_loc*TCH+tc] = cos(b=c*B_LOC+b_loc, t=tc*128+p)
    cos = np.stack([np.asarray(out[c]["cos_stat"], np.float64)
                    for c in range(N_CORES)])
    cos = cos.reshape(N_CORES, P, B_LOC, TCH)
    row_cos = cos.sum(axis=(1, 3)).reshape(B)

    v_acc = np.zeros((P, TCH * D), np.float64)
    a_acc = np.zeros((P, TCH * D), np.float64)
    for c in range(N_CORES):
        v_acc += np.asarray(out[c]["v_acc"], np.float64)
        a_acc += np.asarray(out[c]["a_acc"], np.float64)
    cross_sum = float((v_acc * a_acc).sum())   # = sum_t V_t . A_t

    pos = labels == 0
    n_pos = int(pos.sum())
    n_neg = B - n_pos

    loss_pos = ALPHA * (n_pos * T - row_cos[pos].sum())
    loss_neg = BETA * row_cos[~pos].sum()
    loss_neg += GAMMA * (cross_sum - row_cos.sum()) / T
    cnt_pos = n_pos * T
    cnt_neg = n_neg * T + B * (B - 1)

    loss = 0.0
    if cnt_pos > 0:
        loss += loss_pos / max(cnt_pos, 1.0)
    if cnt_neg > 0:
        loss += loss_neg / max(cnt_neg, 1.0)
    return np.float32(loss)


# revision 3
# speedup vs baseline: 1.2487x; 1.2487x over previous
"""CMFM loss kernel for Trainium2 (8 NeuronCores, Bass/Tile) — v2.

Math: for inputs f_v, f_a [B,T,D] with vn/an the D-normalized tensors,
  row_cos[b]  = sum_t cos(vn[b,t],an[b,t])   (per-timestep term)
  sum_{i!=j} cross = (1/T)*(sum_t V_t.A_t - sum_b row_cos[b])
where V_t = sum_b vn[b,t,:].  Data-parallel over B (8 rows/core); host
sums the 8 per-core V/A partials and dots them.

Design notes (every cost HW-measured on this stack):
  * t-mapping t = p*8 + j (partition-major): each partition's DRAM
    slice is 8KB contiguous, so a casting SWDGE DMA needs 128
    descriptors/MB instead of 2048 — Q7 descriptor emission was the
    v2.0 bottleneck.  All outputs are t-sums or consistently-indexed
    pairs, so the relabeling needs no host-side compensation.
  * bf16 on-chip via casting DMAs (fp32 HBM -> bf16 SBUF, ~440GB/s).
  * TensorE does normalize-and-accumulate: per (b,chunk) a diagonal
    stationary diag(inv) [128x128] bf16 times the moving tile
    [128,256] accumulates V += inv_v*v in PSUM (111ns/chunk incl.
    LDW).  V uses 4 PSUM banks, A the other 4; each bank is
    has_written-cleared once by a zero-stationary dummy matmul (the
    start=True clear is bank-wide, so half-bank groups can't use it).
  * Diagonals built 8-at-a-time with a double-broadcast tensor op
    (ident[p,f] * inv[p,c]): DVE 1431ns/row (v), GpSimd 1971ns/row (a).
  * s_vv/s_aa: ACT batched Square (257/tile-eq) + DVE halve-add (2x
    mode, 102/t-e) + tensor_reduce (171/t-e); K_FV/K_FA trailing
    chunks instead use fused ACT Square+accum (750/chunk) to balance
    ACT vs DVE.
  * row_cos: prod = v*a (DVE TT 2x), halve-add, then one stt with a
    broadcast inv_v*inv_a operand and accum_out -> per-partition
    per-row cosine sums [P, B_LOC].  No per-t cos output needed.
"""

import os as _os

import numpy as np

import concourse.bacc as bacc
import concourse.tile as tile
from concourse import mybir
from concourse.bass_utils import run_bass_kernel_spmd

ALPHA, BETA, GAMMA = 2.0, 2.0, 1.0
B, T, D = 64, 1024, 256
N_CORES = 8
B_LOC = B // N_CORES          # 8 batch rows per core
P = 128                       # SBUF partitions
TCH = T // P                  # 8 t-chunks per batch row
HD = D // 2                   # 128, halve-add width

F32 = mybir.dt.float32
BF16 = mybir.dt.bfloat16
MULT = mybir.AluOpType.mult
ADD = mybir.AluOpType.add

# knobs
K_FV = int(_os.environ.get("K_FV", "3"))   # fused ACT sq+accum chunks/row (v)
K_FA = int(_os.environ.get("K_FA", "3"))   # fused chunks/row (a)
K_DIAG = _os.environ.get("K_DIAG", "split")  # 'dve' | 'gps' | 'split'
K_IO_BUFS = int(_os.environ.get("K_IO_BUFS", "2"))
K_SCR_BUFS = int(_os.environ.get("K_SCR_BUFS", "2"))
K_ROWS_PER_DMA = int(_os.environ.get("K_ROWS_PER_DMA", "4"))

_CACHE = {}
LAST_RESULTS = None


def _build_nc(repeat=1, loop_n=1):
    nc = bacc.Bacc("TRN2", debug=False)
    RPD = K_ROWS_PER_DMA

    v = nc.dram_tensor("v", [B_LOC, T, D], F32, kind="ExternalInput").ap()
    a = nc.dram_tensor("a", [B_LOC, T, D], F32, kind="ExternalInput").ap()
    rc_out = nc.dram_tensor("rc_stat", [P, B_LOC], F32, kind="ExternalOutput").ap()
    vacc_out = nc.dram_tensor("v_acc", [P, TCH * D], F32, kind="ExternalOutput").ap()
    aacc_out = nc.dram_tensor("a_acc", [P, TCH * D], F32, kind="ExternalOutput").ap()

    with tile.TileContext(nc) as tc:
        with (
            tc.tile_pool(name="io", bufs=K_IO_BUFS) as iop,
            tc.tile_pool(name="scr", bufs=K_SCR_BUFS) as scr,
            tc.tile_pool(name="small", bufs=K_SCR_BUFS + 1) as small,
            tc.tile_pool(name="const", bufs=1) as cst,
            tc.tile_pool(name="ps", bufs=1, space="PSUM") as psp,
        ):
            # --- persistent/constant tiles ---
            ident = cst.tile([P, P], BF16)
            zeros512 = cst.tile([P, 4, HD], BF16)   # moving operand for dummy-clears
            rc_stat = cst.tile([P, B_LOC], F32)
            psV = psp.tile([P, TCH, D], F32)        # 4 banks
            psA = psp.tile([P, TCH, D], F32)        # 4 banks

            iota_p = cst.tile([P, 1], F32)
            iota_f = cst.tile([P, P], F32)
            nc.gpsimd.iota(iota_p[:], pattern=[[1, 1]], base=0,
                           channel_multiplier=1,
                           allow_small_or_imprecise_dtypes=True)
            nc.gpsimd.iota(iota_f[:], pattern=[[1, P]], base=0,
                           channel_multiplier=0,
                           allow_small_or_imprecise_dtypes=True)
            nc.vector.tensor_scalar(out=ident[:], in0=iota_f[:],
                                    scalar1=iota_p[:, 0:1], scalar2=None,
                                    op0=mybir.AluOpType.is_equal)
            nc.gpsimd.memset(zeros512[:], 0.0)

            ident_bc = ident[:].rearrange("p (o q) -> p o q", o=1) \
                               .broadcast_to([P, TCH, P])

            import contextlib
            loop_ctx = (
                tc.For_i(
                    0, loop_n, 1,
                    hint_engines=(
                        mybir.EngineType.DVE,
                        mybir.EngineType.Activation,
                        mybir.EngineType.SP,
                        mybir.EngineType.PE,
                    ),
                )
                if loop_n > 1
                else contextlib.nullcontext()
            )
            with loop_ctx:
              for _ in range(repeat):
                # dummy matmuls: zero stationary, start=True clears each
                # bank's has_written bits so the per-chunk groups accumulate
                for k in range(4):
                    nc.tensor.matmul(out=psV[:, 2 * k:2 * k + 2, :],
                                     lhsT=zeros512[:, 0, :], rhs=zeros512[:],
                                     start=True, stop=False,
                                     skip_group_check=True)
                    nc.tensor.matmul(out=psA[:, 2 * k:2 * k + 2, :],
                                     lhsT=zeros512[:, 0, :], rhs=zeros512[:],
                                     start=True, stop=False,
                                     skip_group_check=True)

                supers = None
                for b in range(B_LOC):
                    if b % RPD == 0:
                        vtg = iop.tile([P, RPD, TCH, D], BF16, tag="vt")
                        atg = iop.tile([P, RPD, TCH, D], BF16, tag="at")
                        # t = p*TCH + j: per (p,b) the DRAM slice is 8KB
                        # contiguous -> 128 descriptors per row per tensor
                        vr = v[b:b + RPD].rearrange("b (p j) d -> p b j d", p=P)
                        ar = a[b:b + RPD].rearrange("b (p j) d -> p b j d", p=P)
                        nc.gpsimd.dma_start(out=vtg[:], in_=vr)
                        nc.gpsimd.dma_start(out=atg[:], in_=ar)
                        supers = (vtg, atg)
                    vtg, atg = supers
                    vt = vtg[:, b % RPD]     # [P, TCH, D] bf16
                    at = atg[:, b % RPD]

                    pair = small.tile([P, 2 * TCH], F32, tag="pair")
                    dump = scr.tile([P, D], BF16, tag="dump")

                    # --- s_vv / s_aa ---
                    for (x, nf, col) in ((vt, K_FV, 0), (at, K_FA, 1)):
                        nb = TCH - nf      # batched chunks [0, nb)
                        sq = scr.tile([P, TCH, D], BF16, tag=f"sq{col}")
                        if nb > 0:
                            nc.scalar.activation(
                                out=sq[:, 0:nb, :], in_=x[:, 0:nb, :],
                                func=mybir.ActivationFunctionType.Square)
                            hv = scr.tile([P, TCH, HD], BF16, tag=f"hv{col}")
                            nc.vector.tensor_tensor(
                                out=hv[:, 0:nb, :], in0=sq[:, 0:nb, 0:HD],
                                in1=sq[:, 0:nb, HD:D], op=ADD)
                            nc.vector.tensor_reduce(
                                out=pair[:, col * TCH:col * TCH + nb],
                                in_=hv[:, 0:nb, :],
                                axis=mybir.AxisListType.X, op=ADD)
                        for c in range(nb, TCH):
                            nc.scalar.activation(
                                out=dump[:], in_=x[:, c, :],
                                func=mybir.ActivationFunctionType.Square,
                                accum_out=pair[:, col * TCH + c:col * TCH + c + 1])

                    # --- norms -> inv ---
                    norm = small.tile([P, 2 * TCH], F32, tag="norm")
                    inv_f = small.tile([P, 2 * TCH], F32, tag="invf")
                    inv_b = small.tile([P, 2 * TCH], BF16, tag="invb")
                    iva_b = small.tile([P, TCH], BF16, tag="ivab")
                    nc.scalar.activation(out=norm[:], in_=pair[:],
                                         func=mybir.ActivationFunctionType.Sqrt)
                    nc.vector.reciprocal(out=inv_f[:], in_=norm[:])
                    nc.vector.tensor_copy(out=inv_b[:], in_=inv_f[:])
                    nc.vector.tensor_mul(out=iva_b[:], in0=inv_f[:, 0:TCH],
                                         in1=inv_f[:, TCH:2 * TCH])

                    # --- row-cos: prod, halve, bcast-scale with accum ---
                    prod = scr.tile([P, TCH, D], BF16, tag="prod")
                    nc.vector.tensor_tensor(out=prod[:], in0=vt, in1=at,
                                            op=MULT)
                    hp = scr.tile([P, TCH, HD], BF16, tag="hp")
                    nc.vector.tensor_tensor(out=hp[:], in0=prod[:, :, 0:HD],
                                            in1=prod[:, :, HD:D], op=ADD)
                    iva_bc = iva_b[:].rearrange("p (c o) -> p c o", o=1) \
                                     .broadcast_to([P, TCH, HD])
                    rcs = scr.tile([P, TCH, HD], BF16, tag="rcs")
                    nc.vector.scalar_tensor_tensor(
                        out=rcs[:], in0=hp[:], scalar=1.0, in1=iva_bc,
                        op0=MULT, op1=MULT,
                        accum_out=rc_stat[:, b:b + 1])

                    # --- diagonals: dg[p, c, f] = ident[p, f] * inv[p, c]
                    dgv = scr.tile([P, TCH, P], BF16, tag="dgv")
                    dga = scr.tile([P, TCH, P], BF16, tag="dga")
                    ibv = inv_b[:, 0:TCH].rearrange("p (c o) -> p c o", o=1) \
                        .broadcast_to([P, TCH, P])
                    iba = inv_b[:, TCH:2 * TCH] \
                        .rearrange("p (c o) -> p c o", o=1) \
                        .broadcast_to([P, TCH, P])
                    dg_v_eng = nc.vector if K_DIAG in ("dve", "split") else nc.gpsimd
                    dg_a_eng = nc.gpsimd if K_DIAG in ("gps", "split") else nc.vector
                    dg_v_eng.tensor_tensor(out=dgv[:], in0=ident_bc, in1=ibv,
                                           op=MULT)
                    dg_a_eng.tensor_tensor(out=dga[:], in0=ident_bc, in1=iba,
                                           op=MULT)

                    # --- PE: V/A accumulate, per chunk ---
                    last = (b == B_LOC - 1)
                    for c in range(TCH):
                        nc.tensor.matmul(out=psV[:, c, :], lhsT=dgv[:, c, :],
                                         rhs=vt[:, c, :], start=False,
                                         stop=last, skip_group_check=True)
                        nc.tensor.matmul(out=psA[:, c, :], lhsT=dga[:, c, :],
                                         rhs=at[:, c, :], start=False,
                                         stop=last, skip_group_check=True)

                # --- epilogue ---
                nc.sync.dma_start(out=rc_out, in_=rc_stat[:])
                vacc_sb = cst.tile([P, TCH, D], F32)
                aacc_sb = cst.tile([P, TCH, D], F32)
                nc.vector.tensor_copy(out=vacc_sb[:], in_=psV[:])
                nc.vector.tensor_copy(out=aacc_sb[:], in_=psA[:])
                nc.sync.dma_start(out=vacc_out, in_=vacc_sb[:])
                nc.sync.dma_start(out=aacc_out, in_=aacc_sb[:])

    nc.compile()
    return nc


def _get_nc(repeat=1, loop_n=1):
    key = ("nc", repeat, loop_n, K_FV, K_FA, K_DIAG, K_IO_BUFS, K_SCR_BUFS,
           K_ROWS_PER_DMA)
    if key not in _CACHE:
        _CACHE[key] = _build_nc(repeat, loop_n)
    return _CACHE[key]


def _run(nc, f_v, f_a):
    in_maps = [
        {
            "v": np.ascontiguousarray(f_v[c * B_LOC:(c + 1) * B_LOC]),
            "a": np.ascontiguousarray(f_a[c * B_LOC:(c + 1) * B_LOC]),
        }
        for c in range(N_CORES)
    ]
    return run_bass_kernel_spmd(nc, in_maps, core_ids=list(range(N_CORES)))


def kernel(f_v, f_a, labels):
    global LAST_RESULTS
    f_v = np.asarray(f_v, dtype=np.float32)
    f_a = np.asarray(f_a, dtype=np.float32)
    labels = np.asarray(labels)

    res = _run(_get_nc(), f_v, f_a)
    LAST_RESULTS = res
    out = res.results

    # rc_stat[c][p, b_loc] = sum over this partition's t of cos[b, t]
    rc = np.stack([np.asarray(out[c]["rc_stat"], np.float64)
                   for c in range(N_CORES)])
    row_cos = rc.sum(axis=1).reshape(B)

    v_acc = np.zeros((P, TCH * D), np.float64)
    a_acc = np.zeros((P, TCH * D), np.float64)
    for c in range(N_CORES):
        v_acc += np.asarray(out[c]["v_acc"], np.float64)
        a_acc += np.asarray(out[c]["a_acc"], np.float64)
    cross_sum = float((v_acc * a_acc).sum())   # = sum_t V_t . A_t

    pos = labels == 0
    n_pos = int(pos.sum())
    n_neg = B - n_pos

    loss_pos = ALPHA * (n_pos * T - row_cos[pos].sum())
    loss_neg = BETA * row_cos[~pos].sum()
    loss_neg += GAMMA * (cross_sum - row_cos.sum()) / T
    cnt_pos = n_pos * T
    cnt_neg = n_neg * T + B * (B - 1)

    loss = 0.0
    if cnt_pos > 0:
        loss += loss_pos / max(cnt_pos, 1.0)
    if cnt_neg > 0:
        loss += loss_neg / max(cnt_neg, 1.0)
    return np.float32(loss)


# revision 10
# speedup vs baseline: 2.4534x; 1.9647x over previous
"""CMFM loss kernel for Trainium2 (8 NeuronCores, Bass/Tile) — v2.

Math: for inputs f_v, f_a [B,T,D] with vn/an the D-normalized tensors,
  row_cos[b]  = sum_t cos(vn[b,t],an[b,t])   (per-timestep term)
  sum_{i!=j} cross = (1/T)*(sum_t V_t.A_t - sum_b row_cos[b])
where V_t = sum_b vn[b,t,:].  Data-parallel over B (8 rows/core); host
sums the 8 per-core V/A partials and dots them.

Design notes (every cost HW-measured on this stack):
  * t-mapping t = p*8 + j (partition-major): each partition's DRAM
    slice is 8KB contiguous, so a casting SWDGE DMA needs 128
    descriptors/MB instead of 2048 — Q7 descriptor emission was the
    v2.0 bottleneck.  All outputs are t-sums or consistently-indexed
    pairs, so the relabeling needs no host-side compensation.
  * bf16 on-chip via casting DMAs (fp32 HBM -> bf16 SBUF, ~440GB/s).
  * TensorE does normalize-and-accumulate: per (b,chunk) a diagonal
    stationary diag(inv) [128x128] bf16 times the moving tile
    [128,256] accumulates V += inv_v*v in PSUM (111ns/chunk incl.
    LDW).  V uses 4 PSUM banks, A the other 4; each bank is
    has_written-cleared once by a zero-stationary dummy matmul (the
    start=True clear is bank-wide, so half-bank groups can't use it).
  * Diagonals built 8-at-a-time with a double-broadcast tensor op
    (ident[p,f] * inv[p,c]): DVE 1431ns/row (v), GpSimd 1971ns/row (a).
  * s_vv/s_aa: ACT batched Square (257/tile-eq) + DVE halve-add (2x
    mode, 102/t-e) + tensor_reduce (171/t-e); K_FV/K_FA trailing
    chunks instead use fused ACT Square+accum (750/chunk) to balance
    ACT vs DVE.
  * row_cos: prod = v*a (DVE TT 2x), halve-add, then one stt with a
    broadcast inv_v*inv_a operand and accum_out -> per-partition
    per-row cosine sums [P, B_LOC].  No per-t cos output needed.
  * Loads: one casting DMA per tensor split 1/3/4 rows (SWDGE has ~10us
    per-DMA overhead -> few big DMAs; the small first slice lets compute
    start early).  Outputs v_acc/a_acc in bf16 (halves output DMA).

Measured per-core pass (min-stats slope over For_i repeats): ~71-97us
depending on machine load, vs 110-146us for the v1 all-DVE/ACT kernel
measured the same way.  Loads alone measure ~53us; DVE is the busiest
compute engine.  Relative error vs fp32 reference: 2.4e-06.
"""

import os as _os

import numpy as np

import concourse.bacc as bacc
import concourse.tile as tile
from concourse import mybir
from concourse.bass_utils import run_bass_kernel_spmd

ALPHA, BETA, GAMMA = 2.0, 2.0, 1.0
B, T, D = 64, 1024, 256
N_CORES = 8
B_LOC = B // N_CORES          # 8 batch rows per core
P = 128                       # SBUF partitions
TCH = T // P                  # 8 t-chunks per batch row
HD = D // 2                   # 128, halve-add width

F32 = mybir.dt.float32
BF16 = mybir.dt.bfloat16
MULT = mybir.AluOpType.mult
ADD = mybir.AluOpType.add

# knobs
K_FV = int(_os.environ.get("K_FV", "3"))   # fused ACT sq+accum chunks/row (v)
K_FA = int(_os.environ.get("K_FA", "3"))   # fused chunks/row (a)
K_DIAG = _os.environ.get("K_DIAG", "split")  # 'dve' | 'gps' | 'split'
K_IO_BUFS = int(_os.environ.get("K_IO_BUFS", "2"))
K_SCR_BUFS = int(_os.environ.get("K_SCR_BUFS", "2"))
K_ROWS_PER_DMA = int(_os.environ.get("K_ROWS_PER_DMA", "8"))
# DMA split within each row-group: e.g. "2,6" issues the group's rows as
# two DMAs of 2 and 6 rows so compute can start after the first lands.
K_SPLIT = _os.environ.get("K_SPLIT", "1,3,4")

_CACHE = {}
LAST_RESULTS = None


def _build_nc(repeat=1, loop_n=1):
    nc = bacc.Bacc("TRN2", debug=False)
    RPD = K_ROWS_PER_DMA

    v = nc.dram_tensor("v", [B_LOC, T, D], F32, kind="ExternalInput").ap()
    a = nc.dram_tensor("a", [B_LOC, T, D], F32, kind="ExternalInput").ap()
    rc_out = nc.dram_tensor("rc_stat", [P, B_LOC], F32, kind="ExternalOutput").ap()
    vacc_out = nc.dram_tensor("v_acc", [P, TCH * D], BF16, kind="ExternalOutput").ap()
    aacc_out = nc.dram_tensor("a_acc", [P, TCH * D], BF16, kind="ExternalOutput").ap()

    with tile.TileContext(nc) as tc:
        with (
            tc.tile_pool(name="io", bufs=K_IO_BUFS) as iop,
            tc.tile_pool(name="scr", bufs=K_SCR_BUFS) as scr,
            tc.tile_pool(name="small", bufs=K_SCR_BUFS + 1) as small,
            tc.tile_pool(name="const", bufs=1) as cst,
            tc.tile_pool(name="ps", bufs=1, space="PSUM") as psp,
        ):
            # --- persistent/constant tiles ---
            ident = cst.tile([P, P], BF16)
            zeros512 = cst.tile([P, 4, HD], BF16)   # moving operand for dummy-clears
            rc_stat = cst.tile([P, B_LOC], F32)
            psV = psp.tile([P, TCH, D], F32)        # 4 banks
            psA = psp.tile([P, TCH, D], F32)        # 4 banks

            iota_p = cst.tile([P, 1], F32)
            iota_f = cst.tile([P, P], F32)
            nc.gpsimd.iota(iota_p[:], pattern=[[1, 1]], base=0,
                           channel_multiplier=1,
                           allow_small_or_imprecise_dtypes=True)
            nc.gpsimd.iota(iota_f[:], pattern=[[1, P]], base=0,
                           channel_multiplier=0,
                           allow_small_or_imprecise_dtypes=True)
            nc.vector.tensor_scalar(out=ident[:], in0=iota_f[:],
                                    scalar1=iota_p[:, 0:1], scalar2=None,
                                    op0=mybir.AluOpType.is_equal)
            nc.gpsimd.memset(zeros512[:], 0.0)

            ident_bc = ident[:].rearrange("p (o q) -> p o q", o=1) \
                               .broadcast_to([P, TCH, P])

            import contextlib
            loop_ctx = (
                tc.For_i(
                    0, loop_n, 1,
                    hint_engines=(
                        mybir.EngineType.DVE,
                        mybir.EngineType.Activation,
                        mybir.EngineType.SP,
                        mybir.EngineType.PE,
                    ),
                )
                if loop_n > 1
                else contextlib.nullcontext()
            )
            with loop_ctx:
              for _ in range(repeat):
                # dummy matmuls: zero stationary, start=True clears each
                # bank's has_written bits so the per-chunk groups accumulate
                for k in range(4):
                    nc.tensor.matmul(out=psV[:, 2 * k:2 * k + 2, :],
                                     lhsT=zeros512[:, 0, :], rhs=zeros512[:],
                                     start=True, stop=False,
                                     skip_group_check=True)
                    nc.tensor.matmul(out=psA[:, 2 * k:2 * k + 2, :],
                                     lhsT=zeros512[:, 0, :], rhs=zeros512[:],
                                     start=True, stop=False,
                                     skip_group_check=True)

                supers = None
                for b in range(B_LOC):
                    if b % RPD == 0:
                        vtg = iop.tile([P, RPD, TCH, D], BF16, tag="vt")
                        atg = iop.tile([P, RPD, TCH, D], BF16, tag="at")
                        # t = p*TCH + j: per (p,b) the DRAM slice is 8KB
                        # contiguous -> 128 descriptors per row per tensor
                        vr = v[b:b + RPD].rearrange("b (p j) d -> p b j d", p=P)
                        ar = a[b:b + RPD].rearrange("b (p j) d -> p b j d", p=P)
                        splits = ([int(s) for s in K_SPLIT.split(",")]
                                  if K_SPLIT else [RPD])
                        assert sum(splits) == RPD
                        o = 0
                        for w in splits:
                            nc.gpsimd.dma_start(out=vtg[:, o:o + w],
                                                in_=vr[:, o:o + w])
                            nc.gpsimd.dma_start(out=atg[:, o:o + w],
                                                in_=ar[:, o:o + w])
                            o += w
                        supers = (vtg, atg)
                    vtg, atg = supers
                    vt = vtg[:, b % RPD]     # [P, TCH, D] bf16
                    at = atg[:, b % RPD]

                    pair = small.tile([P, 2 * TCH], F32, tag="pair")
                    dump = scr.tile([P, D], BF16, tag="dump")

                    # --- s_vv / s_aa ---
                    for (x, nf, col) in ((vt, K_FV, 0), (at, K_FA, 1)):
                        nb = TCH - nf      # batched chunks [0, nb)
                        sq = scr.tile([P, TCH, D], BF16, tag=f"sq{col}")
                        if nb > 0:
                            nc.scalar.activation(
                                out=sq[:, 0:nb, :], in_=x[:, 0:nb, :],
                                func=mybir.ActivationFunctionType.Square)
                            hv = scr.tile([P, TCH, HD], BF16, tag=f"hv{col}")
                            nc.vector.tensor_tensor(
                                out=hv[:, 0:nb, :], in0=sq[:, 0:nb, 0:HD],
                                in1=sq[:, 0:nb, HD:D], op=ADD)
                            nc.vector.tensor_reduce(
                                out=pair[:, col * TCH:col * TCH + nb],
                                in_=hv[:, 0:nb, :],
                                axis=mybir.AxisListType.X, op=ADD)
                        for c in range(nb, TCH):
                            nc.scalar.activation(
                                out=dump[:], in_=x[:, c, :],
                                func=mybir.ActivationFunctionType.Square,
                                accum_out=pair[:, col * TCH + c:col * TCH + c + 1])

                    # --- norms -> inv ---
                    norm = small.tile([P, 2 * TCH], F32, tag="norm")
                    inv_f = small.tile([P, 2 * TCH], F32, tag="invf")
                    inv_b = small.tile([P, 2 * TCH], BF16, tag="invb")
                    iva_b = small.tile([P, TCH], BF16, tag="ivab")
                    nc.scalar.activation(out=norm[:], in_=pair[:],
                                         func=mybir.ActivationFunctionType.Sqrt)
                    nc.vector.reciprocal(out=inv_f[:], in_=norm[:])
                    nc.vector.tensor_copy(out=inv_b[:], in_=inv_f[:])
                    nc.vector.tensor_mul(out=iva_b[:], in0=inv_f[:, 0:TCH],
                                         in1=inv_f[:, TCH:2 * TCH])

                    # --- row-cos: prod, halve, bcast-scale with accum ---
                    prod = scr.tile([P, TCH, D], BF16, tag="prod")
                    nc.vector.tensor_tensor(out=prod[:], in0=vt, in1=at,
                                            op=MULT)
                    hp = scr.tile([P, TCH, HD], BF16, tag="hp")
                    nc.vector.tensor_tensor(out=hp[:], in0=prod[:, :, 0:HD],
                                            in1=prod[:, :, HD:D], op=ADD)
                    iva_bc = iva_b[:].rearrange("p (c o) -> p c o", o=1) \
                                     .broadcast_to([P, TCH, HD])
                    rcs = scr.tile([P, TCH, HD], BF16, tag="rcs")
                    nc.vector.scalar_tensor_tensor(
                        out=rcs[:], in0=hp[:], scalar=1.0, in1=iva_bc,
                        op0=MULT, op1=MULT,
                        accum_out=rc_stat[:, b:b + 1])

                    # --- diagonals: dg[p, c, f] = ident[p, f] * inv[p, c]
                    dgv = scr.tile([P, TCH, P], BF16, tag="dgv")
                    dga = scr.tile([P, TCH, P], BF16, tag="dga")
                    ibv = inv_b[:, 0:TCH].rearrange("p (c o) -> p c o", o=1) \
                        .broadcast_to([P, TCH, P])
                    iba = inv_b[:, TCH:2 * TCH] \
                        .rearrange("p (c o) -> p c o", o=1) \
                        .broadcast_to([P, TCH, P])
                    dg_v_eng = nc.vector if K_DIAG in ("dve", "split") else nc.gpsimd
                    dg_a_eng = nc.gpsimd if K_DIAG in ("gps", "split") else nc.vector
                    dg_v_eng.tensor_tensor(out=dgv[:], in0=ident_bc, in1=ibv,
                                           op=MULT)
                    dg_a_eng.tensor_tensor(out=dga[:], in0=ident_bc, in1=iba,
                                           op=MULT)

                    # --- PE: V/A accumulate, per chunk ---
                    last = (b == B_LOC - 1)
                    for c in range(TCH):
                        nc.tensor.matmul(out=psV[:, c, :], lhsT=dgv[:, c, :],
                                         rhs=vt[:, c, :], start=False,
                                         stop=last, skip_group_check=True)
                        nc.tensor.matmul(out=psA[:, c, :], lhsT=dga[:, c, :],
                                         rhs=at[:, c, :], start=False,
                                         stop=last, skip_group_check=True)

                # --- epilogue: per-bank evac (bf16) so DMAs overlap copies ---
                nc.sync.dma_start(out=rc_out, in_=rc_stat[:])
                vacc_sb = cst.tile([P, TCH, D], BF16)
                aacc_sb = cst.tile([P, TCH, D], BF16)
                vo = vacc_out.rearrange("p (c d) -> p c d", d=D)
                ao = aacc_out.rearrange("p (c d) -> p c d", d=D)
                for k in range(4):
                    sl = slice(2 * k, 2 * k + 2)
                    nc.vector.tensor_copy(out=vacc_sb[:, sl, :], in_=psV[:, sl, :])
                    nc.scalar.activation(out=aacc_sb[:, sl, :], in_=psA[:, sl, :],
                                         func=mybir.ActivationFunctionType.Copy)
                    nc.sync.dma_start(out=vo[:, sl, :], in_=vacc_sb[:, sl, :])
                    nc.sync.dma_start(out=ao[:, sl, :], in_=aacc_sb[:, sl, :])

    nc.compile()
    return nc


def _get_nc(repeat=1, loop_n=1):
    key = ("nc", repeat, loop_n, K_FV, K_FA, K_DIAG, K_IO_BUFS, K_SCR_BUFS,
           K_ROWS_PER_DMA, K_SPLIT)
    if key not in _CACHE:
        _CACHE[key] = _build_nc(repeat, loop_n)
    return _CACHE[key]


def _run(nc, f_v, f_a):
    in_maps = [
        {
            "v": np.ascontiguousarray(f_v[c * B_LOC:(c + 1) * B_LOC]),
            "a": np.ascontiguousarray(f_a[c * B_LOC:(c + 1) * B_LOC]),
        }
        for c in range(N_CORES)
    ]
    return run_bass_kernel_spmd(nc, in_maps, core_ids=list(range(N_CORES)))


def kernel(f_v, f_a, labels):
    global LAST_RESULTS
    f_v = np.asarray(f_v, dtype=np.float32)
    f_a = np.asarray(f_a, dtype=np.float32)
    labels = np.asarray(labels)

    res = _run(_get_nc(), f_v, f_a)
    LAST_RESULTS = res
    out = res.results

    # rc_stat[c][p, b_loc] = sum over this partition's t of cos[b, t]
    rc = np.stack([np.asarray(out[c]["rc_stat"], np.float64)
                   for c in range(N_CORES)])
    row_cos = rc.sum(axis=1).reshape(B)

    v_acc = np.zeros((P, TCH * D), np.float64)
    a_acc = np.zeros((P, TCH * D), np.float64)
    for c in range(N_CORES):
        v_acc += np.asarray(out[c]["v_acc"], np.float64)
        a_acc += np.asarray(out[c]["a_acc"], np.float64)
    cross_sum = float((v_acc * a_acc).sum())   # = sum_t V_t . A_t

    pos = labels == 0
    n_pos = int(pos.sum())
    n_neg = B - n_pos

    loss_pos = ALPHA * (n_pos * T - row_cos[pos].sum())
    loss_neg = BETA * row_cos[~pos].sum()
    loss_neg += GAMMA * (cross_sum - row_cos.sum()) / T
    cnt_pos = n_pos * T
    cnt_neg = n_neg * T + B * (B - 1)

    loss = 0.0
    if cnt_pos > 0:
        loss += loss_pos / max(cnt_pos, 1.0)
    if cnt_neg > 0:
        loss += loss_neg / max(cnt_neg, 1.0)
    return np.float32(loss)
